# revision 29
# baseline (speedup 1.0000x reference)
"""Decoder layer (attn + FFN + 2 layernorms) on 8 Trainium2 cores — v2.

Sharding: core c handles batch b = c//4, query chunk i = c%4 (512 tokens).
Each core redundantly computes K/V for the full sequence (communication-free).
Causality: key/value token order is rotated per core on the host (self chunk
first, then past, then future) so the mask structure is uniform across cores:
k-tiles 0-3 (self) get host-built triangular bf16 masks, the rest a per-core
additive bias (0 past, -1e30 future) folded into the softmax exp. Softmax is
unnormalized (scores O(+-8)); the denominator comes from a ones-column
appended to V and is divided out of the accumulated context.

v2 vs v1: everything bf16 (weights, x, activations) — converted and laid out
on the HOST, so no on-device f32r rounding passes and no PE transposes; K/V
computed once, flat, fully SBUF-resident; per-head context accumulates across
all 16 k-tiles directly in PSUM; the FFN intermediate (d_ff=4096, bf16) stays
in SBUF instead of bouncing through DRAM; PSUM->SBUF copies are spread across
Scalar/Vector/Pool engines.
"""

import sys

sys.path.insert(0, "/opt/trn_rl_repo")

import numpy as np

D = 1024          # d_model
H = 16            # heads
HD = 64           # head dim
DFF = 4096
EPS = 1e-6
B, S = 2, 2048
QCH = 512         # query tokens per core
NCORES = 8
P = 128
KT = S // P               # 16 k tiles of 128 tokens
NDT = D // P              # 8 d_model tiles
NFT = DFF // P            # 32 d_ff tiles
NEG = -1.0e30

_CACHE = {}


def _build(debug=False):
    import concourse.bacc as bacc
    import concourse.mybir as mybir
    import concourse.tile as tile

    dt = mybir.dt
    BF = dt.bfloat16
    AF = mybir.ActivationFunctionType
    OP = mybir.AluOpType

    nc = bacc.Bacc("TRN2", target_bir_lowering=False, debug=False)

    # ---- I/O (all host-pre-laid-out; bf16 for matmul operands) ----
    xT = nc.dram_tensor("xT", [P, NDT, S], BF, kind="ExternalInput")
    kbias = nc.dram_tensor("kbias", [P, KT], dt.float32, kind="ExternalInput")
    tri = nc.dram_tensor("tri", [P, 4, 2 * QCH], BF, kind="ExternalInput")
    onesr = nc.dram_tensor("onesr", [P, P], BF, kind="ExternalInput")
    wq = nc.dram_tensor("wq", [P, NDT, D], BF, kind="ExternalInput")
    wk = nc.dram_tensor("wk", [P, NDT, D], BF, kind="ExternalInput")
    wv = nc.dram_tensor("wv", [P, NDT, D], BF, kind="ExternalInput")
    wo = nc.dram_tensor("wo", [P, NDT, D], BF, kind="ExternalInput")
    w1 = nc.dram_tensor("w1", [P, NDT, DFF], BF, kind="ExternalInput")
    w2 = nc.dram_tensor("w2", [P, NFT, D], BF, kind="ExternalInput")
    bq = nc.dram_tensor("bq", [P, NDT], dt.float32, kind="ExternalInput")
    bk = nc.dram_tensor("bk", [P, NDT], dt.float32, kind="ExternalInput")
    bvb = nc.dram_tensor("bvb", [P, D], BF, kind="ExternalInput")
    bo = nc.dram_tensor("bo", [P, NDT], dt.float32, kind="ExternalInput")
    b1 = nc.dram_tensor("b1", [P, NFT], dt.float32, kind="ExternalInput")
    b2 = nc.dram_tensor("b2", [P, NDT], dt.float32, kind="ExternalInput")
    g1 = nc.dram_tensor("g1", [P, NDT], dt.float32, kind="ExternalInput")
    be1 = nc.dram_tensor("be1", [P, NDT], dt.float32, kind="ExternalInput")
    g2 = nc.dram_tensor("g2", [P, NDT], dt.float32, kind="ExternalInput")
    be2 = nc.dram_tensor("be2", [P, NDT], dt.float32, kind="ExternalInput")
    out = nc.dram_tensor("out", [P, NDT, QCH], dt.float32, kind="ExternalOutput")

    with tile.TileContext(nc) as tc:
        with (
            tc.tile_pool(name="consts", bufs=1) as consts,
            tc.tile_pool(name="wbig", bufs=2) as wbig,
            tc.tile_pool(name="mid", bufs=1) as mid,
            tc.tile_pool(name="expp", bufs=4) as expp,
            tc.tile_pool(name="small", bufs=2) as small,
        ):
            # ---- constants (bulk ones deferred below the gating DMAs) ----
            kbias_sb = consts.tile([P, KT], dt.float32, tag="kbias")
            nc.sync.dma_start(kbias_sb[:], kbias[:])
            eps_sb = consts.tile([P, 1], dt.float32, tag="eps")
            nc.vector.memset(eps_sb[:], EPS)

            def load_pd(name, ap, n):
                t = consts.tile([P, n], dt.float32, tag=name, name=name)
                nc.sync.dma_start(t[:], ap[:])
                return t

            bq_sb = load_pd("bq", bq, NDT)
            bk_sb = load_pd("bk", bk, NDT)
            bo_sb = load_pd("bo", bo, NDT)
            b1_sb = load_pd("b1", b1, NFT)
            b2_sb = load_pd("b2", b2, NDT)
            g1_sb = load_pd("g1", g1, NDT)
            be1_sb = load_pd("be1", be1, NDT)
            g2_sb = load_pd("g2", g2, NDT)
            be2_sb = load_pd("be2", be2, NDT)

            def wtile(src_ap, name, nsplit=1):
                t = wbig.tile([P, NDT, D], BF, tag="w", name=name)
                if nsplit == 1:
                    nc.sync.dma_start(t[:], src_ap)
                else:  # split across DMA queues for parallel bandwidth
                    step = NDT // nsplit
                    for s in range(nsplit):
                        nc.sync.dma_start(t[:, s * step:(s + 1) * step, :],
                                          src_ap[:, s * step:(s + 1) * step, :])
                return t

            def layer_norm(ps_pool, src, dst, g_sb, be_sb, out_dma=None):
                """dst[:, do, :] = LN(src) over d_model (partition + do axes);
                per-token (free-axis) stats via ones-matmul column sums."""
                ps1 = ps_pool.tile([P, QCH], dt.float32, tag="ln", name="ps1")
                for do in range(NDT):
                    nc.tensor.matmul(ps1[:], onesr_sb[:], src[:, do, :],
                                     start=(do == 0), stop=(do == NDT - 1))
                ps2 = ps_pool.tile([P, QCH], dt.float32, tag="ln", name="ps2")
                for do in range(NDT):
                    sq = small.tile([P, QCH], BF, tag="sq")
                    nc.vector.tensor_tensor(sq[:], src[:, do, :], src[:, do, :],
                                            OP.mult)
                    nc.tensor.matmul(ps2[:], onesr_sb[:], sq[:],
                                     start=(do == 0), stop=(do == NDT - 1))
                mean = small.tile([P, QCH], BF, tag="mean")
                nc.vector.tensor_scalar(out=mean[:], in0=ps1[:], scalar1=1.0 / D,
                                        scalar2=None, op0=OP.mult)
                m2 = small.tile([P, QCH], BF, tag="m2")
                nc.vector.tensor_tensor(m2[:], mean[:], mean[:], OP.mult)
                var = small.tile([P, QCH], BF, tag="var")
                nc.vector.scalar_tensor_tensor(
                    out=var[:], in0=ps2[:], scalar=1.0 / D, in1=m2[:],
                    op0=OP.mult, op1=OP.subtract)
                sstd = small.tile([P, QCH], BF, tag="sstd")
                nc.scalar.activation(out=sstd[:], in_=var[:], func=AF.Sqrt,
                                     bias=eps_sb[:], scale=1.0)
                rstd = small.tile([P, QCH], BF, tag="rstd")
                with nc.allow_low_precision(reason="bf16 rstd, ~4e-3 rel ok"):
                    nc.vector.reciprocal(out=rstd[:], in_=sstd[:])
                for do in range(NDT):
                    # odd-do chains go to the otherwise-idle Pool engine so
                    # the LN tail isn't serialized on the Vector engine
                    eng = nc.vector if do % 2 == 0 else nc.gpsimd
                    t1 = small.tile([P, QCH], BF, tag=f"ln_t1{do % 2}",
                                    name="t1")
                    eng.tensor_tensor(t1[:], src[:, do, :], mean[:],
                                      OP.subtract)
                    eng.tensor_tensor(t1[:], t1[:], rstd[:], OP.mult)
                    eng.tensor_scalar(
                        out=dst[:, do, :], in0=t1[:],
                        scalar1=g_sb[:, do:do + 1], scalar2=be_sb[:, do:do + 1],
                        op0=OP.mult, op1=OP.add)
                    if out_dma is not None:
                        nc.sync.dma_start(out_dma[:, do, :], dst[:, do, :])

            ctxT = mid.tile([P, NDT, QCH], BF, tag="ctxT")
            yT = mid.tile([P, NDT, QCH], BF, tag="yT")
            hT = mid.tile([P, NDT, QCH], BF, tag="hT")

            with tc.tile_pool(name="attn", bufs=1) as attn:
                xT_sb = attn.tile([P, NDT, S], BF, tag="xT")
                for s in range(4):
                    nc.sync.dma_start(xT_sb[:, 2 * s:2 * s + 2, 0:QCH],
                                      xT[:, 2 * s:2 * s + 2, 0:QCH])
                kt_sb = attn.tile([P, NDT, S], BF, tag="kt")
                v_sb = attn.tile([P, KT, H, HD + 1], BF, tag="v")
                qT_sb = attn.tile([P, NDT, QCH], BF, tag="qT")
                nc.vector.memset(v_sb[:, :, :, HD], 1.0)

                # ---- projections ----
                with (
                    tc.tile_pool(name="psP", bufs=2, space="PSUM") as psP,
                ):
                    wq_t = wtile(wq[:], "wq_t", nsplit=4)
                    for s in range(4):
                        nc.sync.dma_start(xT_sb[:, 2 * s:2 * s + 2, QCH:],
                                          xT[:, 2 * s:2 * s + 2, QCH:])
                    for do in range(NDT):
                        pq = psP.tile([P, QCH], dt.float32, tag="pq")
                        for k in range(NDT):
                            nc.tensor.matmul(
                                pq[:], wq_t[:, k, do * P:(do + 1) * P],
                                xT_sb[:, k, 0:QCH],
                                start=(k == 0), stop=(k == NDT - 1))
                        nc.vector.tensor_scalar(
                            out=qT_sb[:, do, :], in0=pq[:],
                            scalar1=bq_sb[:, do:do + 1], scalar2=None,
                            op0=OP.add)
                    wk_t = wtile(wk[:], "wk_t", nsplit=4)
                    for do in range(NDT):
                        for np_ in range(2):
                            pk = psP.tile([P, 2 * QCH], dt.float32, tag="pk")
                            for half in range(2):
                                n = 2 * np_ + half
                                for k in range(NDT):
                                    nc.tensor.matmul(
                                        pk[:, half * QCH:(half + 1) * QCH],
                                        wk_t[:, k, do * P:(do + 1) * P],
                                        xT_sb[:, k, n * QCH:(n + 1) * QCH],
                                        start=(k == 0), stop=(k == NDT - 1))
                            nc.scalar.activation(
                                out=kt_sb[:, do, np_ * 2 * QCH:(np_ + 1) * 2 * QCH],
                                in_=pk[:], func=AF.Identity,
                                bias=bk_sb[:, do:do + 1], scale=1.0)
                    wv_t = wtile(wv[:], "wv_t", nsplit=2)
                    # bulk constants — needed from attention onwards
                    tri_sb = consts.tile([P, 4, 2 * QCH], BF, tag="tri")
                    nc.sync.dma_start(tri_sb[:], tri[:])
                    onesr_sb = consts.tile([P, P], BF, tag="onesr")
                    nc.sync.dma_start(onesr_sb[:], onesr[:])
                    bvb_sb = consts.tile([P, D], BF, tag="bvb")
                    nc.sync.dma_start(bvb_sb[:], bvb[:])
                    for tt in range(KT):
                        pv = psP.tile([P, QCH], dt.float32, tag="pq",
                                      name="pv")
                        for k in range(NDT):
                            nc.tensor.matmul(
                                pv[:], xT_sb[:, k, tt * P:(tt + 1) * P],
                                wv_t[:, k, 0:QCH],
                                start=(k == 0), stop=(k == NDT - 1))
                        nc.vector.tensor_tensor(
                            v_sb[:, tt, 0:8, 0:HD],
                            pv[:].rearrange("p (h d) -> p h d", d=HD),
                            bvb_sb[:, 0:QCH].rearrange(
                                "p (h d) -> p h d", d=HD),
                            OP.add)

                # ---- attention: 8 head pairs, ctx accumulates in PSUM ----
                wo_t = wtile(wo[:], "wo_t")  # prefetch during attention
                with (
                    tc.tile_pool(name="psS", bufs=3, space="PSUM") as psS,
                    tc.tile_pool(name="psC", bufs=2, space="PSUM") as psC,
                ):
                    for a in range(H // 2):
                        pcs = [psC.tile([P, QCH], dt.float32, tag="pc",
                                        name=f"pc{i}") for i in range(2)]

                        def emit_scores(j, a=a):
                            psc = psS.tile([P, 2 * QCH], dt.float32,
                                           tag="psc", name="psc")
                            for i in range(2):
                                bp = i * HD
                                nc.tensor.matmul(
                                    psc[:, i * QCH:(i + 1) * QCH],
                                    kt_sb[bp:bp + HD, a, j * P:(j + 1) * P],
                                    qT_sb[bp:bp + HD, a, :],
                                    start=True, stop=True,
                                    tile_position=(bp, 0))
                            return psc

                        psc_cur = emit_scores(0)
                        for j in range(KT):
                            # software pipeline: next j's scores go to the PE
                            # ahead of this j's ctx so the PE never waits on
                            # the Scalar engine's exp
                            psc_next = emit_scores(j + 1) if j + 1 < KT else None
                            ex = expp.tile([P, 2 * QCH], BF, tag="exp")
                            nc.scalar.activation(
                                out=ex[:], in_=psc_cur[:], func=AF.Exp,
                                bias=kbias_sb[:, j:j + 1], scale=0.125)
                            if j < 4:
                                nc.vector.tensor_tensor(ex[:], ex[:],
                                                        tri_sb[:, j, :],
                                                        OP.mult)
                            for i in range(2):
                                nc.tensor.matmul(
                                    pcs[i][0:HD + 1, :], v_sb[:, j, 2 * a + i, :],
                                    ex[:, i * QCH:(i + 1) * QCH],
                                    start=(j == 0), stop=(j == KT - 1))
                            psc_cur = psc_next
                            # second half of the V projection (heads 8-15),
                            # interleaved into the PE stream of pairs 0-3
                            if a < 4 and j % 4 == 3:
                                tt = 4 * a + j // 4
                                pvw = psS.tile([P, 2 * QCH], dt.float32,
                                               tag="psc", name="pvw")
                                pv1 = pvw[:, 0:QCH]
                                for k in range(NDT):
                                    nc.tensor.matmul(
                                        pv1[:], xT_sb[:, k, tt * P:(tt + 1) * P],
                                        wv_t[:, k, QCH:2 * QCH],
                                        start=(k == 0), stop=(k == NDT - 1))
                                nc.vector.tensor_tensor(
                                    v_sb[:, tt, 8:16, 0:HD],
                                    pv1[:].rearrange("p (h d) -> p h d", d=HD),
                                    bvb_sb[:, QCH:2 * QCH].rearrange(
                                        "p (h d) -> p h d", d=HD),
                                    OP.add)
                        # copy raw ctx + recip out of PSUM fast (frees the
                        # pcs banks for the next pair), then normalize
                        # in-place off the PE's critical path
                        rcs = []
                        for i in range(2):
                            rc = small.tile([1, QCH], BF, tag=f"rc{i}",
                                            name="rc")
                            with nc.allow_low_precision(
                                    reason="bf16 recip colsum, ~4e-3 ok"):
                                nc.vector.reciprocal(out=rc[:],
                                                     in_=pcs[i][HD:HD + 1, :])
                            nc.vector.tensor_copy(
                                out=ctxT[i * HD:(i + 1) * HD, a, :],
                                in_=pcs[i][0:HD, :])
                            rcs.append(rc)
                        for i in range(2):
                            prcb = small.tile([P, QCH], BF, tag="prcb",
                                              name="prcb")
                            nc.gpsimd.partition_broadcast(prcb[:], rcs[i][:])
                            nc.vector.tensor_tensor(
                                ctxT[i * HD:(i + 1) * HD, a, :],
                                ctxT[i * HD:(i + 1) * HD, a, :],
                                prcb[i * HD:(i + 1) * HD, :], OP.mult)

                # ---- O proj + residual + LN1 ----
                w1q = [None] * 4
                w1q[0] = wtile(w1[:, :, 0:D], "w1q")  # prefetch
                with tc.tile_pool(name="psO", bufs=2, space="PSUM") as psO:
                    for do in range(NDT):
                        po = psO.tile([P, QCH], dt.float32, tag="po")
                        for k in range(NDT):
                            nc.tensor.matmul(
                                po[:], wo_t[:, k, do * P:(do + 1) * P],
                                ctxT[:, k, :],
                                start=(k == 0), stop=(k == NDT - 1))
                        nc.vector.scalar_tensor_tensor(
                            out=yT[:, do, :], in0=po[:],
                            scalar=bo_sb[:, do:do + 1],
                            in1=xT_sb[:, do, 0:QCH], op0=OP.add, op1=OP.add)
                    layer_norm(psO, yT, hT, g1_sb, be1_sb)

            # ---- FFN (intermediate stays in SBUF, bf16) ----
            with tc.tile_pool(name="ffnp", bufs=1) as ffnp:
                ff_sb = ffnp.tile([P, NFT, QCH], BF, tag="ff")
                y2T = ffnp.tile([P, NDT, QCH], BF, tag="y2T")
                outT = ffnp.tile([P, NDT, QCH], dt.float32, tag="outT")
                w2qs = {}
                with tc.tile_pool(name="psF1", bufs=4, space="PSUM") as psF1:
                    for ft in range(NFT):
                        if ft % 8 == 0 and ft // 8 < 3:
                            q = ft // 8 + 1
                            w1q[q] = wtile(w1[:, :, q * D:(q + 1) * D], "w1q")
                        if ft == 16:  # prefetch first W2 quarter
                            w2qs[0] = wbig.tile([P, NDT, D], BF, tag="w",
                                                name="w2q")
                            nc.sync.dma_start(w2qs[0][:], w2[:, 0:8, :])
                        pf = psF1.tile([P, QCH], dt.float32, tag="pf")
                        wt = w1q[ft // 8]
                        for k in range(NDT):
                            nc.tensor.matmul(
                                pf[:], wt[:, k, (ft % 8) * P:(ft % 8 + 1) * P],
                                hT[:, k, :],
                                start=(k == 0), stop=(k == NDT - 1))
                        nc.scalar.activation(
                            out=ff_sb[:, ft, :], in_=pf[:], func=AF.Relu,
                            bias=b1_sb[:, ft:ft + 1], scale=1.0)
                with tc.tile_pool(name="psF2", bufs=8, space="PSUM") as psF2:
                    accs = [psF2.tile([P, QCH], dt.float32, tag="acc",
                                      name=f"acc{do}") for do in range(NDT)]
                    for k in range(NFT):
                        if k % 8 == 0 and k > 0:
                            w2qs[k // 8] = wbig.tile([P, NDT, D], BF, tag="w",
                                                     name="w2q")
                            nc.sync.dma_start(w2qs[k // 8][:],
                                              w2[:, k:k + 8, :])
                        w2q = w2qs[k // 8]
                        for do in range(NDT):
                            nc.tensor.matmul(
                                accs[do][:], w2q[:, k % 8, do * P:(do + 1) * P],
                                ff_sb[:, k, :],
                                start=(k == 0), stop=(k == NFT - 1))
                    for do in range(NDT):
                        nc.vector.scalar_tensor_tensor(
                            out=y2T[:, do, :], in0=accs[do][:],
                            scalar=b2_sb[:, do:do + 1], in1=hT[:, do, :],
                            op0=OP.add, op1=OP.add)
                with tc.tile_pool(name="psL2", bufs=2, space="PSUM") as psL2:
                    layer_norm(psL2, y2T, outT, g2_sb, be2_sb, out_dma=out)

    nc.finalize()
    return nc


def _get_nc(debug=False):
    key = ("nc", debug)
    if key not in _CACHE:
        _CACHE[key] = _build(debug)
    return _CACHE[key]


def _prep_shared(Wq, bq, Wk, bk, Wv, bv, Wo, bo, W1, b1, W2, b2,
                 gamma1, beta1, gamma2, beta2):
    import ml_dtypes
    BF = ml_dtypes.bfloat16

    def wT(W):  # [D, N] -> [P, D//P, N] (d_in split over partitions)
        W = np.asarray(W, np.float32)
        kt = W.shape[0] // P
        return np.ascontiguousarray(
            W.reshape(kt, P, W.shape[1]).transpose(1, 0, 2).astype(BF))

    def pd(v):  # [N] -> [P, N//P] (per-partition layout, fp32)
        v = np.asarray(v, np.float32)
        return np.ascontiguousarray(v.reshape(-1, P).T)

    # triangular self-chunk masks, duplicated across the two 512 halves
    p_ = np.arange(P)[:, None]
    f_ = np.arange(QCH)[None, :]
    tri = np.zeros((P, 4, 2 * QCH), np.float32)
    for j in range(4):
        m = ((P * j + p_) <= f_).astype(np.float32)
        tri[:, j, 0:QCH] = m
        tri[:, j, QCH:] = m
    return {
        "wq": wT(Wq), "wk": wT(Wk), "wv": wT(Wv), "wo": wT(Wo),
        "w1": wT(W1), "w2": wT(W2),
        "bq": pd(bq), "bk": pd(bk), "bo": pd(bo),
        "b1": pd(b1), "b2": pd(b2),
        "g1": pd(gamma1), "be1": pd(beta1), "g2": pd(gamma2), "be2": pd(beta2),
        "bvb": np.ascontiguousarray(
            np.broadcast_to(np.asarray(bv, np.float32), (P, D)).astype(BF)),
        "tri": tri.astype(BF),
        "onesr": np.ones((P, P), BF),
    }


def kernel(x, mask, Wq, bq, Wk, bk, Wv, bv, Wo, bo, W1, b1, W2, b2,
           gamma1, beta1, gamma2, beta2, _trace=False, _debug=False,
           _mm_dtype=None):
    import ml_dtypes
    from concourse.bass_utils import run_bass_kernel_spmd

    BF = ml_dtypes.bfloat16
    nc = _get_nc(_debug)
    x = np.ascontiguousarray(np.asarray(x, dtype=np.float32))
    shared = _prep_shared(Wq, bq, Wk, bk, Wv, bv, Wo, bo, W1, b1, W2, b2,
                          gamma1, beta1, gamma2, beta2)
    in_maps = []
    for c in range(NCORES):
        b, i = divmod(c, NCORES // B)
        q0 = i * QCH
        xb_rot = np.concatenate(
            [x[b, q0:q0 + QCH], x[b, :q0], x[b, q0 + QCH:]], axis=0)
        # [S, D] -> [P, NDT, S] transposed layout, bf16
        xTc = xb_rot.T.reshape(NDT, P, S).transpose(1, 0, 2).astype(BF)
        kb = np.zeros((P, KT), np.float32)
        n_ok = 4 + q0 // P  # self tiles + past tiles
        kb[:, n_ok:] = NEG
        in_maps.append({
            "xT": np.ascontiguousarray(xTc),
            "kbias": kb,
            **shared,
        })
    res = run_bass_kernel_spmd(nc, in_maps, core_ids=list(range(NCORES)),
                               trace=_trace)
    out = np.empty((B, S, D), np.float32)
    for c in range(NCORES):
        b, i = divmod(c, NCORES // B)
        r = np.asarray(res.results[c]["out"], np.float32)  # [P, NDT, QCH]
        out[b, i * QCH:(i + 1) * QCH] = (
            r.transpose(2, 1, 0).reshape(QCH, D))
    if _trace:
        _CACHE["last_result"] = res
    return out
